# revision 22
# baseline (speedup 1.0000x reference)
"""TRN2 Bass kernel for nn_BSNLayer (batched spectral-norm-like layer).

Math (per batch element):
    X = x.reshape(C, HW)                      # C=512, HW=4096
    Ws = X @ X.T / HW                         # scaled Gram matrix, (C, C)
    w  = Ws^10 @ v0  (unnormalized power-iteration direction)
    u' = X.T @ w ;  beta = 1/(||w|| * ||u'||) = rsqrt(w.w * HW * w.Ws.w)
    out = x + beta * outer(w, u').reshape(C, H, W)

Kernel strategy (8 cores, 2 batch elements per core, pure data parallel):
  - bf16 end to end: host casts x to bf16; output is produced TRANSPOSED
    ([HW, C] bf16) and the host transposes/upcasts. Halves DMA volume,
    which is the hard floor of this problem.
  - Only X^T ever exists in SBUF, loaded straight from DRAM with
    DmaTranspose (xbar tiles) -> no PE transposes, no PSUM evacuations
    for X at all.
  - Gram: upper-triangular strips (N=512/384/256/128) accumulated over
    32 hw-chunks; lower blocks reconstructed with bf16 PE transposes.
  - Power iteration: 10 chained matvecs (N=1 matmuls + PSUM->SBUF bf16
    evac). Batch 0's chain + row-broadcast are interleaved into batch
    1's gram matmul stream so PE never idles and the tail overlaps.
  - u' = X^T w per hw-tile via DVE scalar_tensor_tensor accum_out
    against the broadcast w row; out^T tile = XT + u'[p]*(beta*w_row)
    fused in one more STT; stores fire per 4-chunk group.
  - beta scalar chain runs on Pool/ACT so the DVE STT stream is never
    interrupted; PE is pre-warmed with dummy matmuls so the gram runs
    at full clock from the first chunk.
"""

import numpy as np
import ml_dtypes

import concourse.bass as bass
import concourse.mybir as mybir
import concourse.tile as tile
from concourse import masks
from concourse.bass_utils import run_bass_kernel_spmd

F32 = mybir.dt.float32
BF16 = mybir.dt.bfloat16
MULT = mybir.AluOpType.mult
ADD = mybir.AluOpType.add
RSQRT = mybir.ActivationFunctionType.Rsqrt

N_CORES = 8
B_FULL, C, H, W = 16, 512, 64, 64
HW = H * W
BPC = B_FULL // N_CORES
P = 128
CT = C // P       # 4 c-tiles
KT = HW // P      # 32 hw chunks of 128
BLOCKS = (2, 3, 5, 6, 8, 8)   # hw-chunks per DMA-transpose load block
SBLK = 8          # store blocks per batch
KPS = KT // SBLK  # hw chunks per store block (4)
GRAM_SCALE = 1.0 / HW
IP = 10
WARM_MMS = 10
# element-wise lane assignment (tuned against TimelineSim).
# ACT carries no bulk element-wise work for batch 0: it is the low-latency
# lane (gram evacs, sym copies, chain evacs) that gates the next phase.
U_POOL_KS0 = frozenset(k for k in range(KT) if k % 2 == 1)
OUT_LANES0_EARLY = ("dve", "pool")                  # blocks 0-3, under gram1
OUT_LANES0_LATE = ("act", "dve", "act", "act", "dve", "act", "dve", "act")
U_POOL_KS1 = frozenset(k for k in range(KT) if k % 16 in (1, 3, 5, 7, 9, 11, 13))
OUT_LANES1 = ("act", "dve", "pool", "act", "dve", "act", "pool", "dve")


class ChunkedDrainTileContext(tile.TileContext):
    """TileContext whose tail drain splits its sem waits across several SP
    drains -- the stock single Drain exceeds this walrus build's
    per-instruction sync-command limit."""

    def _drain_and_barrier(self, tick_clock, wait_clock):
        from concourse.vector_clock import ScopedClock, VectorClock

        gc = tick_clock.global_clock
        n = len(gc)
        procs = [i for i in range(n) if gc[i] > 0]
        for p in procs:
            vc = VectorClock([gc[j] if j == p else 0 for j in range(n)])
            fan_inst = self.nc.sync.drain(fusable=False)
            wait_clock.add_sem_waits(fan_inst.ins, ScopedClock({None: vc}))
        self.nc.sync.drain()

        self.nc.all_engine_barrier()
        assert self.sems is not None
        popped = self.nc._tile_sem_poison_stack.pop()
        assert popped is self._sem_poison
        self.nc.clear_and_free_semaphores(list(self.sems.allocated().values()))
        self.nc.all_engine_barrier()


def _split_excess_waits(nc, keep=1):
    """This walrus build allows only ~2 sync commands per instruction (and 1
    for no-ctrl-struct ops). Keep at most `keep` waits on each instruction and
    move the rest onto injected single-wait NoOps just before it (same
    engine, so queue order preserves wait semantics)."""
    n = 0
    for fn in nc.m.functions:
        for blk in fn.blocks:
            out = []
            changed = False
            for inst in blk.instructions:
                si = inst.sync_info
                if si is not None:
                    waits = list(si.on_wait or [])
                    ups = list(si.on_update or [])
                    if len(waits) > keep:
                        for w in waits[:-keep]:
                            nop = mybir.InstNoOp(name=f"wsplit{n}", ins=[],
                                                 outs=[])
                            n += 1
                            nop.engine = inst.engine
                            nop.sync_info = mybir.SyncInfo(on_wait=[w],
                                                           on_update=[])
                            out.append(nop)
                        inst.sync_info = mybir.SyncInfo(on_wait=waits[-keep:],
                                                        on_update=ups)
                        changed = True
                out.append(inst)
            if changed:
                blk.instructions = out
    return nc


class _St:
    pass


def build():
    nc = bass.Bass("TRN2", target_bir_lowering=False, debug=False,
                   num_devices=N_CORES)
    x_d = nc.dram_tensor("x", [BPC, C, HW], BF16, kind="ExternalInput").ap()
    v_d = nc.dram_tensor("v", [BPC, C, 1], F32, kind="ExternalInput").ap()
    o_d = nc.dram_tensor("out", [BPC, HW, C], BF16, kind="ExternalOutput").ap()

    with ChunkedDrainTileContext(nc) as tc:
        with tc.tile_pool(name="pconst", bufs=1) as pc, \
             tc.tile_pool(name="pxt", bufs=1) as pxt, \
             tc.tile_pool(name="pws", bufs=1) as pws, \
             tc.tile_pool(name="psm", bufs=2) as psm, \
             tc.tile_pool(name="pscr", bufs=3) as pscr, \
             tc.tile_pool(name="pp", bufs=1, space="PSUM") as pp:

            identf = pc.tile([P, P], F32, name="identf")
            masks.make_identity(nc, identf[:])
            identb = pc.tile([P, P], BF16, name="identb")
            nc.vector.tensor_copy(identb[:], identf[:])
            ones_col = pc.tile([P, 1], F32, name="ones_col")
            nc.vector.memset(ones_col[:], 1.0)
            ones_row_b = pc.tile([1, P], BF16, name="ones_row_b")
            nc.vector.memset(ones_row_b[:], 1.0)
            ones_row_f = pc.tile([1, P], F32, name="ones_row_f")
            nc.vector.memset(ones_row_f[:], 1.0)
            warm = pc.tile([P, C], BF16, name="warm")
            nc.vector.memset(warm[:], 0.5)

            sts = [_St() for _ in range(BPC)]

            # ---------------- loads (all on SP so the DMA device serves
            # b0's blocks strictly before b1's -- two queues interleave and
            # starve b0's gram) ----------------------------------------
            for b in range(BPC):
                st = sts[b]
                q = nc.sync
                st.xt = []
                k0 = 0
                for r, nk in enumerate(BLOCKS):
                    t = pxt.tile([P, nk * C], BF16, tag=f"xt{b}_{r}",
                                 name=f"xt_{b}_{r}")
                    st.xt.append((k0, t))
                    q.dma_start_transpose(
                        t[:, :].rearrange("p (a c) -> p a c", c=C),
                        x_d[b, :, k0 * P:(k0 + nk) * P])
                    k0 += nk
                st.v0 = psm.tile([P, CT], F32, tag=f"v0_{b}", name=f"v0_{b}")
                q.dma_start(st.v0[:],
                            v_d[b].rearrange("(a p) o -> p (a o)", p=P))
                st.vb = psm.tile([P, CT], BF16, tag=f"vb_{b}", bufs=3,
                                 name=f"vb_{b}")
                nc.vector.tensor_copy(st.vb[:], st.v0[:])

            def xts(st, k, lo, hi):
                for k0, t in st.xt:
                    nk = t.shape[1] // C
                    if k0 <= k < k0 + nk:
                        a = k - k0
                        return t[:, a * C + lo:a * C + hi]
                raise AssertionError(k)

            # ---------------- PE warmup (runs while first block loads) ---
            wps = pp.tile([P, C], F32, tag="mv", bufs=2, name="warm_ps")
            for i in range(WARM_MMS):
                nc.tensor.matmul(wps[:], identb[:], warm[:],
                                 start=(i == 0), stop=(i == WARM_MMS - 1),
                                 skip_group_check=True)

            # ---------------- gram helpers ------------------------------
            gCsh = pp.tile([P, C], F32, tag="gc", bufs=1, name="gCsh")

            def emit_gram_mms(b, interleave=None):
                st = sts[b]
                gA = pp.tile([P, C], F32, tag="g", bufs=3, name=f"gA_{b}")
                gB = pp.tile([P, C], F32, tag="g", bufs=3, name=f"gB_{b}")
                st.gps = [gA[:, 0:C], gB[:, 0:384],
                          gCsh[:, b * 256:(b + 1) * 256], gB[:, 384:C]]
                for k in range(KT):
                    for i in range(CT):
                        nc.tensor.matmul(
                            st.gps[i][:, :],
                            xts(st, k, i * P, (i + 1) * P),
                            xts(st, k, i * P, C),
                            start=(k == 0 and i != 3),
                            stop=(k == KT - 1 and i != 1),
                            skip_group_check=True,
                        )
                    if interleave is not None:
                        interleave(k)

            def emit_gram_evac(b, engs=("act",)):
                st = sts[b]
                st.ws = [pws.tile([P, C], BF16, tag=f"ws{b}_{i}",
                                  name=f"ws_{b}_{i}") for i in range(CT)]
                for i in range(CT):
                    e = engs[i % len(engs)]
                    if e == "act":
                        nc.scalar.mul(st.ws[i][:, i * P:C], st.gps[i][:, :],
                                      GRAM_SCALE)
                    else:
                        nc.vector.tensor_scalar(st.ws[i][:, i * P:C],
                                                st.gps[i][:, :], GRAM_SCALE,
                                                None, op0=MULT)

            def emit_sym(b, pairs, copy_eng="act"):
                st = sts[b]
                for (i, j) in pairs:
                    tp = pp.tile([P, P], BF16, tag="misc", bufs=2,
                                 name=f"sym_{b}_{i}_{j}")
                    nc.tensor.matmul(tp[:], st.ws[i][:, j * P:(j + 1) * P],
                                     identb[:], is_transpose=True,
                                     start=True, stop=True,
                                     skip_group_check=True)
                    if copy_eng == "act":
                        nc.scalar.copy(st.ws[j][:, i * P:(i + 1) * P], tp[:])
                    else:
                        nc.gpsimd.tensor_copy(st.ws[j][:, i * P:(i + 1) * P],
                                              tp[:])

            SYM_PAIRS = [(i, j) for i in range(CT) for j in range(i + 1, CT)]

            # ---------------- power-iteration chain ---------------------
            def chain_step(b, t, evac_eng):
                st = sts[b]
                sp = pp.tile([P, CT], F32, tag="mv", bufs=2,
                             name=f"mv_{b}_{t}")
                for i in range(CT):
                    for kk in range(CT):
                        nc.tensor.matmul(sp[:, i:i + 1],
                                         st.ws[kk][:, i * P:(i + 1) * P],
                                         st.vb[:, kk:kk + 1],
                                         start=(kk == 0), stop=(kk == CT - 1),
                                         skip_group_check=True)
                vb2 = psm.tile([P, CT], BF16, tag=f"vb_{b}", bufs=3,
                               name=f"vb_{b}_{t}")
                if evac_eng == "act":
                    nc.scalar.copy(vb2[:], sp[:])
                else:
                    nc.vector.tensor_copy(vb2[:], sp[:])
                st.vb = vb2
                if t == IP - 1:
                    st.w = vb2

            # ---------------- w row broadcast (PE bits) -----------------
            def emit_wrow(b):
                st = sts[b]
                w = st.w
                wrow_ps = pp.tile([1, C], BF16, tag="misc", bufs=2,
                                  name=f"wrowp_{b}")
                for j in range(CT):
                    nc.tensor.matmul(wrow_ps[0:1, j * P:(j + 1) * P],
                                     w[:, j:j + 1], identb[:],
                                     is_transpose=True, start=True, stop=True,
                                     skip_group_check=True)
                st.wrow = psm.tile([1, C], BF16, tag=f"wrow_{b}",
                                   name=f"wrow_{b}")
                nc.scalar.copy(st.wrow[:], wrow_ps[:])

            def emit_vbc(b, evac="dve"):
                st = sts[b]
                vbc_ps = pp.tile([P, C], F32, tag="misc", bufs=2,
                                 name=f"vbcp_{b}")
                nc.tensor.matmul(vbc_ps[:], ones_row_b[0:1, :],
                                 st.wrow[0:1, :], start=True, stop=True,
                                 skip_group_check=True)
                st.vbc = psm.tile([P, C], BF16, tag=f"vbc_{b}",
                                  name=f"vbc_{b}")
                if evac == "act":
                    nc.scalar.copy(st.vbc[:], vbc_ps[:])
                else:
                    nc.vector.tensor_copy(st.vbc[:], vbc_ps[:])
                st.ucols = psm.tile([P, KT], F32, tag=f"uc_{b}",
                                    name=f"uc_{b}")

            # ---------------- beta scalar chain (Pool/ACT/PE) -----------
            def emit_beta_a(b):
                # s4 = Ws_s w (PE); pp1 = rowsum(w*w) (Pool)
                st = sts[b]
                st.s4 = pp.tile([P, CT], F32, tag="mv", bufs=2,
                                name=f"s4_{b}")
                for i in range(CT):
                    for kk in range(CT):
                        nc.tensor.matmul(st.s4[:, i:i + 1],
                                         st.ws[kk][:, i * P:(i + 1) * P],
                                         st.w[:, kk:kk + 1],
                                         start=(kk == 0), stop=(kk == CT - 1),
                                         skip_group_check=True)
                st.t1 = psm.tile([P, CT], F32, tag=f"t1_{b}", name=f"t1_{b}")
                st.pp1 = psm.tile([P, 1], F32, tag=f"pp1_{b}",
                                  name=f"pp1_{b}")
                nc.gpsimd.scalar_tensor_tensor(st.t1[:], st.w[:], 1.0,
                                               st.w[:], op0=MULT, op1=MULT,
                                               accum_out=st.pp1[:])

            def emit_beta_b(b):
                st = sts[b]
                st.t2 = psm.tile([P, CT], F32, tag=f"t2_{b}", name=f"t2_{b}")
                st.pp2 = psm.tile([P, 1], F32, tag=f"pp2_{b}",
                                  name=f"pp2_{b}")
                nc.gpsimd.scalar_tensor_tensor(st.t2[:], st.w[:], 1.0,
                                               st.s4[:], op0=MULT, op1=MULT,
                                               accum_out=st.pp2[:])
                st.d1p = pp.tile([1, 1], F32, tag="misc", bufs=2,
                                 name=f"d1p_{b}")
                nc.tensor.matmul(st.d1p[:], ones_col[:], st.pp1[:],
                                 start=True, stop=True, skip_group_check=True)
                st.d2p = pp.tile([1, 1], F32, tag="misc", bufs=2,
                                 name=f"d2p_{b}")
                nc.tensor.matmul(st.d2p[:], ones_col[:], st.pp2[:],
                                 start=True, stop=True, skip_group_check=True)

            def emit_beta_c(b):
                st = sts[b]
                prod = psm.tile([1, 1], F32, tag=f"prod_{b}",
                                name=f"prod_{b}")
                nc.gpsimd.scalar_tensor_tensor(prod[:], st.d1p[:], float(HW),
                                               st.d2p[:], op0=MULT, op1=MULT)
                binv = psm.tile([1, 1], F32, tag=f"binv_{b}",
                                name=f"binv_{b}")
                nc.scalar.sqrt(binv[:], prod[:])
                beta = psm.tile([1, 1], F32, tag=f"beta_{b}",
                                name=f"beta_{b}")
                nc.vector.reciprocal(beta[:], binv[:])
                bbc_ps = pp.tile([P, 1], F32, tag="misc", bufs=2,
                                 name=f"bbcp_{b}")
                nc.tensor.matmul(bbc_ps[:], ones_row_f[0:1, :], beta[0:1, :],
                                 start=True, stop=True, skip_group_check=True)
                st.bbc = psm.tile([P, 1], F32, tag=f"bbc_{b}",
                                  name=f"bbc_{b}")
                nc.gpsimd.tensor_copy(st.bbc[:], bbc_ps[:])
                st.vbcs = psm.tile([P, C], BF16, tag=f"vbcs_{b}",
                                   name=f"vbcs_{b}")
                nc.gpsimd.tensor_scalar(st.vbcs[:], st.vbc[:], st.bbc[:],
                                        None, op0=MULT)

            # ---------------- u pass + output pass + stores -------------
            # lanes: scalar_tensor_tensor gets NO dve perf mode (1x), so the
            # DVE path uses tensor_tensor (2x) + tensor_scalar+accum (4x);
            # Pool keeps the fused STT (0.6 efficiency), ACT contributes the
            # activation-scale mul of the output pass.
            def emit_upass(b, ks, pool_ks):
                st = sts[b]
                for k in ks:
                    if k in pool_ks:
                        scr = pscr.tile([P, C], BF16, tag="scrp", bufs=2,
                                        name=f"scr_{b}_{k}")
                        nc.gpsimd.scalar_tensor_tensor(
                            scr[:], xts(st, k, 0, C), 1.0, st.vbc[:],
                            op0=MULT, op1=MULT,
                            accum_out=st.ucols[:, k:k + 1])
                    else:
                        scr = pscr.tile([P, C], BF16, tag="scr", bufs=3,
                                        name=f"scr_{b}_{k}")
                        nc.vector.tensor_tensor(scr[:], xts(st, k, 0, C),
                                                st.vbc[:], op=MULT)
                        nc.vector.tensor_scalar(scr[:], scr[:], 1.0, None,
                                                op0=MULT,
                                                accum_out=st.ucols[:, k:k + 1])

            def _group_ap(st, k_first, nk):
                for k0, t in st.xt:
                    tnk = t.shape[1] // C
                    if k0 <= k_first < k0 + tnk:
                        a = k_first - k0
                        assert a + nk <= tnk
                        return t[:, a * C:(a + nk) * C].rearrange(
                            "p (a c) -> p a c", c=C)
                raise AssertionError(k_first)

            def emit_outtile(b, k, lane):
                st = sts[b]
                if lane == "pool":
                    nc.gpsimd.scalar_tensor_tensor(
                        xts(st, k, 0, C), st.vbcs[:],
                        st.ucols[:, k:k + 1], xts(st, k, 0, C),
                        op0=MULT, op1=ADD)
                    return
                zscr = pscr.tile([P, C], BF16, tag="zscr", bufs=3,
                                 name=f"zscr_{b}_{k}")
                if lane == "act":
                    nc.scalar.mul(zscr[:], st.vbcs[:], st.ucols[:, k:k + 1])
                else:
                    nc.vector.tensor_scalar(zscr[:], st.vbcs[:],
                                            st.ucols[:, k:k + 1], None,
                                            op0=MULT)
                nc.vector.tensor_tensor(xts(st, k, 0, C), zscr[:],
                                        xts(st, k, 0, C), op=ADD)

            def emit_outpass(b, lanes, store_qs, blocks=None):
                # store groups == DMA-transpose load blocks (one xt tile each)
                st = sts[b]
                for r in (range(len(BLOCKS)) if blocks is None else blocks):
                    k0, t = st.xt[r]
                    nk = t.shape[1] // C
                    for a in range(nk):
                        emit_outtile(b, k0 + a, lanes[(k0 + a) % len(lanes)])
                    store_qs[r % len(store_qs)].dma_start(
                        o_d[b, k0 * P:(k0 + nk) * P, :]
                        .rearrange("(a p) c -> p a c", p=P),
                        t[:, :].rearrange("p (a c) -> p a c", c=C))

            # ================= schedule =================================
            emit_gram_mms(0)
            emit_gram_evac(0, engs=("act", "dve"))

            # gram1 with b0's sym/chain/wrow/vbc/beta/u interleaved
            ev = {}
            for ki, (i, j) in enumerate(SYM_PAIRS):
                ev.setdefault(ki, []).append(
                    lambda i=i, j=j: emit_sym(0, [(i, j)], "act"))
            for t in range(IP):
                ev.setdefault(6 + t, []).append(
                    lambda t=t: chain_step(0, t, "dve"))
            ev.setdefault(16, []).append(lambda: emit_wrow(0))
            ev.setdefault(17, []).append(lambda: emit_vbc(0, "dve"))
            ev.setdefault(18, []).append(lambda: emit_beta_a(0))
            ev.setdefault(19, []).append(lambda: emit_beta_b(0))
            ev.setdefault(20, []).append(lambda: emit_beta_c(0))
            ev.setdefault(21, []).append(
                lambda: emit_upass(0, range(KT), U_POOL_KS0))
            ev.setdefault(26, []).append(
                lambda: emit_outpass(0, OUT_LANES0_EARLY, [nc.sync],
                                     range(0, 4)))

            def ilv(k):
                for fn in ev.get(k, []):
                    fn()

            emit_gram_mms(1, interleave=ilv)
            emit_gram_evac(1, engs=("act",))
            emit_sym(1, SYM_PAIRS, "act")
            for t in range(IP):
                chain_step(1, t, "act")
            emit_wrow(1)
            emit_vbc(1, "act")
            emit_beta_a(1)
            emit_beta_b(1)
            emit_beta_c(1)
            emit_upass(1, range(KT), U_POOL_KS1)
            emit_outpass(1, OUT_LANES1, [nc.sync, nc.scalar])
            # batch0's trailing output blocks drain last (their stores have
            # DMA slack at the end; they must not delay batch1's u/out pass)
            emit_outpass(0, OUT_LANES0_LATE, [nc.sync], range(4, len(BLOCKS)))

    _split_excess_waits(nc)
    return nc


_NC = None


def kernel(x: np.ndarray, v: np.ndarray) -> np.ndarray:
    global _NC
    assert x.shape == (B_FULL, C, H, W) and v.shape == (B_FULL, C, 1)
    if _NC is None:
        _NC = build()
    xr = np.ascontiguousarray(
        x.reshape(B_FULL, C, HW)).astype(ml_dtypes.bfloat16)
    vr = np.ascontiguousarray(v, dtype=np.float32)
    in_maps = [
        {"x": xr[c * BPC:(c + 1) * BPC], "v": vr[c * BPC:(c + 1) * BPC]}
        for c in range(N_CORES)
    ]
    res = run_bass_kernel_spmd(_NC, in_maps, core_ids=list(range(N_CORES)))
    out_t = np.concatenate([r["out"] for r in res.results], axis=0)
    out = np.transpose(out_t, (0, 2, 1)).astype(np.float32)
    return np.ascontiguousarray(out.reshape(B_FULL, C, H, W))


# revision 24
# speedup vs baseline: 1.0020x; 1.0020x over previous
"""TRN2 Bass kernel for nn_BSNLayer (batched spectral-norm-like layer).

Math (per batch element):
    X = x.reshape(C, HW)                      # C=512, HW=4096
    Ws = X @ X.T / HW                         # scaled Gram matrix, (C, C)
    w  = Ws^10 @ v0  (unnormalized power-iteration direction)
    u' = X.T @ w ;  beta = 1/(||w|| * ||u'||) = rsqrt(w.w * HW * w.Ws.w)
    out = x + beta * outer(w, u').reshape(C, H, W)

Kernel strategy (8 cores, 2 batch elements per core, pure data parallel):
  - bf16 end to end: host casts x to bf16; output is produced TRANSPOSED
    ([HW, C] bf16) and the host transposes/upcasts. Halves DMA volume,
    which is the hard floor of this problem.
  - Only X^T ever exists in SBUF, loaded straight from DRAM with
    DmaTranspose (xbar tiles) -> no PE transposes, no PSUM evacuations
    for X at all.
  - Gram: upper-triangular strips (N=512/384/256/128) accumulated over
    32 hw-chunks; lower blocks reconstructed with bf16 PE transposes.
  - Power iteration: 10 chained matvecs (N=1 matmuls + PSUM->SBUF bf16
    evac). Batch 0's chain + row-broadcast are interleaved into batch
    1's gram matmul stream so PE never idles and the tail overlaps.
  - u' = X^T w per hw-tile via DVE scalar_tensor_tensor accum_out
    against the broadcast w row; out^T tile = XT + u'[p]*(beta*w_row)
    fused in one more STT; stores fire per 4-chunk group.
  - beta scalar chain runs on Pool/ACT so the DVE STT stream is never
    interrupted; PE is pre-warmed with dummy matmuls so the gram runs
    at full clock from the first chunk.
"""

import numpy as np
import ml_dtypes

import concourse.bass as bass
import concourse.mybir as mybir
import concourse.tile as tile
from concourse import masks
from concourse.bass_utils import run_bass_kernel_spmd

F32 = mybir.dt.float32
BF16 = mybir.dt.bfloat16
MULT = mybir.AluOpType.mult
ADD = mybir.AluOpType.add
RSQRT = mybir.ActivationFunctionType.Rsqrt

N_CORES = 8
B_FULL, C, H, W = 16, 512, 64, 64
HW = H * W
BPC = B_FULL // N_CORES
P = 128
CT = C // P       # 4 c-tiles
KT = HW // P      # 32 hw chunks of 128
BLOCKS = (2, 3, 5, 6, 8, 8)   # hw-chunks per DMA-transpose load block
SBLK = 8          # store blocks per batch
KPS = KT // SBLK  # hw chunks per store block (4)
GRAM_SCALE = 1.0 / HW
IP = 10
WARM_MMS = 10
# schedule/lane configuration (tuned against TimelineSim).
# ACT carries no bulk element-wise work for batch 0 during gram1: it is the
# low-latency lane (gram evacs, sym copies, chain evacs) gating phases.
DEFAULT_CFG = dict(
    u_pool0=frozenset(k for k in range(KT) if k % 2 == 1),
    u_pool1=frozenset(k for k in range(KT) if k % 16 in (1, 3, 5, 7, 9, 11, 13)),
    out0_early=("dve", "pool"),
    out0_late=("act", "dve", "act", "act", "dve", "act", "dve", "act"),
    out1=("act", "dve", "pool", "act", "dve", "act", "pool", "dve"),
    hook_chain=6, hook_wrow=16, hook_vbc=17, hook_beta=18,
    hook_upass=21, hook_out0=26,
    evac0=("act", "dve"), evac1=("act",),
    vbc0_evac="dve", vbc1_evac="act",
    out0_early_blocks=4,
)


class ChunkedDrainTileContext(tile.TileContext):
    """TileContext whose tail drain splits its sem waits across several SP
    drains -- the stock single Drain exceeds this walrus build's
    per-instruction sync-command limit."""

    def _drain_and_barrier(self, tick_clock, wait_clock):
        from concourse.vector_clock import ScopedClock, VectorClock

        gc = tick_clock.global_clock
        n = len(gc)
        procs = [i for i in range(n) if gc[i] > 0]
        for p in procs:
            vc = VectorClock([gc[j] if j == p else 0 for j in range(n)])
            fan_inst = self.nc.sync.drain(fusable=False)
            wait_clock.add_sem_waits(fan_inst.ins, ScopedClock({None: vc}))
        self.nc.sync.drain()

        self.nc.all_engine_barrier()
        assert self.sems is not None
        popped = self.nc._tile_sem_poison_stack.pop()
        assert popped is self._sem_poison
        self.nc.clear_and_free_semaphores(list(self.sems.allocated().values()))
        self.nc.all_engine_barrier()


def _split_excess_waits(nc, keep=1):
    """This walrus build allows only ~2 sync commands per instruction (and 1
    for no-ctrl-struct ops). Keep at most `keep` waits on each instruction and
    move the rest onto injected single-wait NoOps just before it (same
    engine, so queue order preserves wait semantics)."""
    n = 0
    for fn in nc.m.functions:
        for blk in fn.blocks:
            out = []
            changed = False
            for inst in blk.instructions:
                si = inst.sync_info
                if si is not None:
                    waits = list(si.on_wait or [])
                    ups = list(si.on_update or [])
                    if len(waits) > keep:
                        for w in waits[:-keep]:
                            nop = mybir.InstNoOp(name=f"wsplit{n}", ins=[],
                                                 outs=[])
                            n += 1
                            nop.engine = inst.engine
                            nop.sync_info = mybir.SyncInfo(on_wait=[w],
                                                           on_update=[])
                            out.append(nop)
                        inst.sync_info = mybir.SyncInfo(on_wait=waits[-keep:],
                                                        on_update=ups)
                        changed = True
                out.append(inst)
            if changed:
                blk.instructions = out
    return nc


class _St:
    pass


def build(cfg=None):
    cfg = dict(DEFAULT_CFG, **(cfg or {}))
    nc = bass.Bass("TRN2", target_bir_lowering=False, debug=False,
                   num_devices=N_CORES)
    x_d = nc.dram_tensor("x", [BPC, C, HW], BF16, kind="ExternalInput").ap()
    v_d = nc.dram_tensor("v", [BPC, C, 1], F32, kind="ExternalInput").ap()
    o_d = nc.dram_tensor("out", [BPC, HW, C], BF16, kind="ExternalOutput").ap()

    with ChunkedDrainTileContext(nc) as tc:
        with tc.tile_pool(name="pconst", bufs=1) as pc, \
             tc.tile_pool(name="pxt", bufs=1) as pxt, \
             tc.tile_pool(name="pws", bufs=1) as pws, \
             tc.tile_pool(name="psm", bufs=2) as psm, \
             tc.tile_pool(name="pscr", bufs=3) as pscr, \
             tc.tile_pool(name="pp", bufs=1, space="PSUM") as pp:

            identf = pc.tile([P, P], F32, name="identf")
            masks.make_identity(nc, identf[:])
            identb = pc.tile([P, P], BF16, name="identb")
            nc.vector.tensor_copy(identb[:], identf[:])
            ones_col = pc.tile([P, 1], F32, name="ones_col")
            nc.vector.memset(ones_col[:], 1.0)
            ones_row_b = pc.tile([1, P], BF16, name="ones_row_b")
            nc.vector.memset(ones_row_b[:], 1.0)
            ones_row_f = pc.tile([1, P], F32, name="ones_row_f")
            nc.vector.memset(ones_row_f[:], 1.0)
            warm = pc.tile([P, C], BF16, name="warm")
            nc.vector.memset(warm[:], 0.5)

            sts = [_St() for _ in range(BPC)]

            # ---------------- loads (all on SP so the DMA device serves
            # b0's blocks strictly before b1's -- two queues interleave and
            # starve b0's gram) ----------------------------------------
            for b in range(BPC):
                st = sts[b]
                q = nc.sync
                st.xt = []
                k0 = 0
                for r, nk in enumerate(BLOCKS):
                    t = pxt.tile([P, nk * C], BF16, tag=f"xt{b}_{r}",
                                 name=f"xt_{b}_{r}")
                    st.xt.append((k0, t))
                    q.dma_start_transpose(
                        t[:, :].rearrange("p (a c) -> p a c", c=C),
                        x_d[b, :, k0 * P:(k0 + nk) * P])
                    k0 += nk
                st.v0 = psm.tile([P, CT], F32, tag=f"v0_{b}", name=f"v0_{b}")
                q.dma_start(st.v0[:],
                            v_d[b].rearrange("(a p) o -> p (a o)", p=P))
                st.vb = psm.tile([P, CT], BF16, tag=f"vb_{b}", bufs=3,
                                 name=f"vb_{b}")
                nc.vector.tensor_copy(st.vb[:], st.v0[:])

            def xts(st, k, lo, hi):
                for k0, t in st.xt:
                    nk = t.shape[1] // C
                    if k0 <= k < k0 + nk:
                        a = k - k0
                        return t[:, a * C + lo:a * C + hi]
                raise AssertionError(k)

            # ---------------- PE warmup (runs while first block loads) ---
            wps = pp.tile([P, C], F32, tag="mv", bufs=2, name="warm_ps")
            for i in range(WARM_MMS):
                nc.tensor.matmul(wps[:], identb[:], warm[:],
                                 start=(i == 0), stop=(i == WARM_MMS - 1),
                                 skip_group_check=True)

            # ---------------- gram helpers ------------------------------
            gCsh = pp.tile([P, C], F32, tag="gc", bufs=1, name="gCsh")

            def emit_gram_mms(b, interleave=None):
                st = sts[b]
                gA = pp.tile([P, C], F32, tag="g", bufs=3, name=f"gA_{b}")
                gB = pp.tile([P, C], F32, tag="g", bufs=3, name=f"gB_{b}")
                st.gps = [gA[:, 0:C], gB[:, 0:384],
                          gCsh[:, b * 256:(b + 1) * 256], gB[:, 384:C]]
                for k in range(KT):
                    for i in range(CT):
                        nc.tensor.matmul(
                            st.gps[i][:, :],
                            xts(st, k, i * P, (i + 1) * P),
                            xts(st, k, i * P, C),
                            start=(k == 0 and i != 3),
                            stop=(k == KT - 1 and i != 1),
                            skip_group_check=True,
                        )
                    if interleave is not None:
                        interleave(k)

            def emit_gram_evac(b, engs=("act",)):
                st = sts[b]
                st.ws = [pws.tile([P, C], BF16, tag=f"ws{b}_{i}",
                                  name=f"ws_{b}_{i}") for i in range(CT)]
                for i in range(CT):
                    e = engs[i % len(engs)]
                    if e == "act":
                        nc.scalar.mul(st.ws[i][:, i * P:C], st.gps[i][:, :],
                                      GRAM_SCALE)
                    else:
                        nc.vector.tensor_scalar(st.ws[i][:, i * P:C],
                                                st.gps[i][:, :], GRAM_SCALE,
                                                None, op0=MULT)

            def emit_sym(b, pairs, copy_eng="act"):
                st = sts[b]
                for (i, j) in pairs:
                    tp = pp.tile([P, P], BF16, tag="misc", bufs=2,
                                 name=f"sym_{b}_{i}_{j}")
                    nc.tensor.matmul(tp[:], st.ws[i][:, j * P:(j + 1) * P],
                                     identb[:], is_transpose=True,
                                     start=True, stop=True,
                                     skip_group_check=True)
                    if copy_eng == "act":
                        nc.scalar.copy(st.ws[j][:, i * P:(i + 1) * P], tp[:])
                    else:
                        nc.gpsimd.tensor_copy(st.ws[j][:, i * P:(i + 1) * P],
                                              tp[:])

            SYM_PAIRS = [(i, j) for i in range(CT) for j in range(i + 1, CT)]

            # ---------------- power-iteration chain ---------------------
            def chain_step(b, t, evac_eng):
                st = sts[b]
                sp = pp.tile([P, CT], F32, tag="mv", bufs=2,
                             name=f"mv_{b}_{t}")
                for i in range(CT):
                    for kk in range(CT):
                        nc.tensor.matmul(sp[:, i:i + 1],
                                         st.ws[kk][:, i * P:(i + 1) * P],
                                         st.vb[:, kk:kk + 1],
                                         start=(kk == 0), stop=(kk == CT - 1),
                                         skip_group_check=True)
                vb2 = psm.tile([P, CT], BF16, tag=f"vb_{b}", bufs=3,
                               name=f"vb_{b}_{t}")
                if evac_eng == "act":
                    nc.scalar.copy(vb2[:], sp[:])
                else:
                    nc.vector.tensor_copy(vb2[:], sp[:])
                st.vb = vb2
                if t == IP - 1:
                    st.w = vb2

            # ---------------- w row broadcast (PE bits) -----------------
            def emit_wrow(b):
                st = sts[b]
                w = st.w
                wrow_ps = pp.tile([1, C], BF16, tag="misc", bufs=2,
                                  name=f"wrowp_{b}")
                for j in range(CT):
                    nc.tensor.matmul(wrow_ps[0:1, j * P:(j + 1) * P],
                                     w[:, j:j + 1], identb[:],
                                     is_transpose=True, start=True, stop=True,
                                     skip_group_check=True)
                st.wrow = psm.tile([1, C], BF16, tag=f"wrow_{b}",
                                   name=f"wrow_{b}")
                nc.scalar.copy(st.wrow[:], wrow_ps[:])

            def emit_vbc(b, evac="dve"):
                st = sts[b]
                vbc_ps = pp.tile([P, C], F32, tag="misc", bufs=2,
                                 name=f"vbcp_{b}")
                nc.tensor.matmul(vbc_ps[:], ones_row_b[0:1, :],
                                 st.wrow[0:1, :], start=True, stop=True,
                                 skip_group_check=True)
                st.vbc = psm.tile([P, C], BF16, tag=f"vbc_{b}",
                                  name=f"vbc_{b}")
                if evac == "act":
                    nc.scalar.copy(st.vbc[:], vbc_ps[:])
                else:
                    nc.vector.tensor_copy(st.vbc[:], vbc_ps[:])
                st.ucols = psm.tile([P, KT], F32, tag=f"uc_{b}",
                                    name=f"uc_{b}")

            # ---------------- beta scalar chain (Pool/ACT/PE) -----------
            def emit_beta_a(b):
                # s4 = Ws_s w (PE); pp1 = rowsum(w*w) (Pool)
                st = sts[b]
                st.s4 = pp.tile([P, CT], F32, tag="mv", bufs=2,
                                name=f"s4_{b}")
                for i in range(CT):
                    for kk in range(CT):
                        nc.tensor.matmul(st.s4[:, i:i + 1],
                                         st.ws[kk][:, i * P:(i + 1) * P],
                                         st.w[:, kk:kk + 1],
                                         start=(kk == 0), stop=(kk == CT - 1),
                                         skip_group_check=True)
                st.t1 = psm.tile([P, CT], F32, tag=f"t1_{b}", name=f"t1_{b}")
                st.pp1 = psm.tile([P, 1], F32, tag=f"pp1_{b}",
                                  name=f"pp1_{b}")
                nc.gpsimd.scalar_tensor_tensor(st.t1[:], st.w[:], 1.0,
                                               st.w[:], op0=MULT, op1=MULT,
                                               accum_out=st.pp1[:])

            def emit_beta_b(b):
                st = sts[b]
                # gpsimd cannot read PSUM: evacuate s4 through ACT first
                s4f = psm.tile([P, CT], F32, tag=f"s4f_{b}", name=f"s4f_{b}")
                nc.scalar.copy(s4f[:], st.s4[:])
                st.t2 = psm.tile([P, CT], F32, tag=f"t2_{b}", name=f"t2_{b}")
                st.pp2 = psm.tile([P, 1], F32, tag=f"pp2_{b}",
                                  name=f"pp2_{b}")
                nc.gpsimd.scalar_tensor_tensor(st.t2[:], st.w[:], 1.0,
                                               s4f[:], op0=MULT, op1=MULT,
                                               accum_out=st.pp2[:])
                st.d1p = pp.tile([1, 1], F32, tag="misc", bufs=2,
                                 name=f"d1p_{b}")
                nc.tensor.matmul(st.d1p[:], ones_col[:], st.pp1[:],
                                 start=True, stop=True, skip_group_check=True)
                st.d2p = pp.tile([1, 1], F32, tag="misc", bufs=2,
                                 name=f"d2p_{b}")
                nc.tensor.matmul(st.d2p[:], ones_col[:], st.pp2[:],
                                 start=True, stop=True, skip_group_check=True)

            def emit_beta_c(b):
                st = sts[b]
                d1c = psm.tile([1, 1], F32, tag=f"d1c_{b}", name=f"d1c_{b}")
                nc.scalar.copy(d1c[:], st.d1p[:])
                d2c = psm.tile([1, 1], F32, tag=f"d2c_{b}", name=f"d2c_{b}")
                nc.scalar.copy(d2c[:], st.d2p[:])
                prod = psm.tile([1, 1], F32, tag=f"prod_{b}",
                                name=f"prod_{b}")
                nc.gpsimd.scalar_tensor_tensor(prod[:], d1c[:], float(HW),
                                               d2c[:], op0=MULT, op1=MULT)
                binv = psm.tile([1, 1], F32, tag=f"binv_{b}",
                                name=f"binv_{b}")
                nc.scalar.sqrt(binv[:], prod[:])
                beta = psm.tile([1, 1], F32, tag=f"beta_{b}",
                                name=f"beta_{b}")
                nc.vector.reciprocal(beta[:], binv[:])
                bbc_ps = pp.tile([P, 1], F32, tag="misc", bufs=2,
                                 name=f"bbcp_{b}")
                nc.tensor.matmul(bbc_ps[:], ones_row_f[0:1, :], beta[0:1, :],
                                 start=True, stop=True, skip_group_check=True)
                st.bbc = psm.tile([P, 1], F32, tag=f"bbc_{b}",
                                  name=f"bbc_{b}")
                nc.scalar.copy(st.bbc[:], bbc_ps[:])
                st.vbcs = psm.tile([P, C], BF16, tag=f"vbcs_{b}",
                                   name=f"vbcs_{b}")
                nc.gpsimd.tensor_scalar(st.vbcs[:], st.vbc[:], st.bbc[:],
                                        None, op0=MULT)

            # ---------------- u pass + output pass + stores -------------
            # lanes: scalar_tensor_tensor gets NO dve perf mode (1x), so the
            # DVE path uses tensor_tensor (2x) + tensor_scalar+accum (4x);
            # Pool keeps the fused STT (0.6 efficiency), ACT contributes the
            # activation-scale mul of the output pass.
            def emit_upass(b, ks, pool_ks):
                st = sts[b]
                for k in ks:
                    if k in pool_ks:
                        scr = pscr.tile([P, C], BF16, tag="scrp", bufs=2,
                                        name=f"scr_{b}_{k}")
                        nc.gpsimd.scalar_tensor_tensor(
                            scr[:], xts(st, k, 0, C), 1.0, st.vbc[:],
                            op0=MULT, op1=MULT,
                            accum_out=st.ucols[:, k:k + 1])
                    else:
                        scr = pscr.tile([P, C], BF16, tag="scr", bufs=3,
                                        name=f"scr_{b}_{k}")
                        nc.vector.tensor_tensor(scr[:], xts(st, k, 0, C),
                                                st.vbc[:], op=MULT)
                        nc.vector.tensor_scalar(scr[:], scr[:], 1.0, None,
                                                op0=MULT,
                                                accum_out=st.ucols[:, k:k + 1])

            def _group_ap(st, k_first, nk):
                for k0, t in st.xt:
                    tnk = t.shape[1] // C
                    if k0 <= k_first < k0 + tnk:
                        a = k_first - k0
                        assert a + nk <= tnk
                        return t[:, a * C:(a + nk) * C].rearrange(
                            "p (a c) -> p a c", c=C)
                raise AssertionError(k_first)

            def emit_outtile(b, k, lane):
                st = sts[b]
                if lane == "pool":
                    nc.gpsimd.scalar_tensor_tensor(
                        xts(st, k, 0, C), st.vbcs[:],
                        st.ucols[:, k:k + 1], xts(st, k, 0, C),
                        op0=MULT, op1=ADD)
                    return
                zscr = pscr.tile([P, C], BF16, tag="zscr", bufs=3,
                                 name=f"zscr_{b}_{k}")
                if lane == "act":
                    nc.scalar.mul(zscr[:], st.vbcs[:], st.ucols[:, k:k + 1])
                else:
                    nc.vector.tensor_scalar(zscr[:], st.vbcs[:],
                                            st.ucols[:, k:k + 1], None,
                                            op0=MULT)
                nc.vector.tensor_tensor(xts(st, k, 0, C), zscr[:],
                                        xts(st, k, 0, C), op=ADD)

            def emit_outpass(b, lanes, store_qs, blocks=None):
                # store groups == DMA-transpose load blocks (one xt tile each)
                st = sts[b]
                for r in (range(len(BLOCKS)) if blocks is None else blocks):
                    k0, t = st.xt[r]
                    nk = t.shape[1] // C
                    for a in range(nk):
                        emit_outtile(b, k0 + a, lanes[(k0 + a) % len(lanes)])
                    store_qs[r % len(store_qs)].dma_start(
                        o_d[b, k0 * P:(k0 + nk) * P, :]
                        .rearrange("(a p) c -> p a c", p=P),
                        t[:, :].rearrange("p (a c) -> p a c", c=C))

            # ================= schedule =================================
            emit_gram_mms(0)
            emit_gram_evac(0, engs=cfg["evac0"])

            # gram1 with b0's sym/chain/wrow/vbc/beta/u interleaved
            ev = {}
            for ki, (i, j) in enumerate(SYM_PAIRS):
                ev.setdefault(ki, []).append(
                    lambda i=i, j=j: emit_sym(0, [(i, j)], "act"))
            for t in range(IP):
                ev.setdefault(cfg["hook_chain"] + t, []).append(
                    lambda t=t: chain_step(0, t, "dve"))
            ev.setdefault(cfg["hook_wrow"], []).append(lambda: emit_wrow(0))
            ev.setdefault(cfg["hook_vbc"], []).append(
                lambda: emit_vbc(0, cfg["vbc0_evac"]))
            hb = cfg["hook_beta"]
            ev.setdefault(hb, []).append(lambda: emit_beta_a(0))
            ev.setdefault(hb + 1, []).append(lambda: emit_beta_b(0))
            ev.setdefault(hb + 2, []).append(lambda: emit_beta_c(0))
            ev.setdefault(cfg["hook_upass"], []).append(
                lambda: emit_upass(0, range(KT), cfg["u_pool0"]))
            ne = cfg["out0_early_blocks"]
            ev.setdefault(cfg["hook_out0"], []).append(
                lambda: emit_outpass(0, cfg["out0_early"], [nc.sync],
                                     range(0, ne)))

            def ilv(k):
                for fn in ev.get(k, []):
                    fn()

            emit_gram_mms(1, interleave=ilv)
            emit_gram_evac(1, engs=cfg["evac1"])
            emit_sym(1, SYM_PAIRS, "act")
            for t in range(IP):
                chain_step(1, t, "act")
            emit_wrow(1)
            emit_vbc(1, cfg["vbc1_evac"])
            emit_beta_a(1)
            emit_beta_b(1)
            emit_beta_c(1)
            emit_upass(1, range(KT), cfg["u_pool1"])
            emit_outpass(1, cfg["out1"], [nc.sync, nc.scalar])
            # batch0's trailing output blocks drain last (their stores have
            # DMA slack at the end; they must not delay batch1's u/out pass)
            emit_outpass(0, cfg["out0_late"], [nc.sync],
                         range(ne, len(BLOCKS)))

    _split_excess_waits(nc)
    return nc


_NC = None


def kernel(x: np.ndarray, v: np.ndarray) -> np.ndarray:
    global _NC
    assert x.shape == (B_FULL, C, H, W) and v.shape == (B_FULL, C, 1)
    if _NC is None:
        _NC = build()
    xr = np.ascontiguousarray(
        x.reshape(B_FULL, C, HW)).astype(ml_dtypes.bfloat16)
    vr = np.ascontiguousarray(v, dtype=np.float32)
    in_maps = [
        {"x": xr[c * BPC:(c + 1) * BPC], "v": vr[c * BPC:(c + 1) * BPC]}
        for c in range(N_CORES)
    ]
    res = run_bass_kernel_spmd(_NC, in_maps, core_ids=list(range(N_CORES)))
    out_t = np.concatenate([r["out"] for r in res.results], axis=0)
    out = np.transpose(out_t, (0, 2, 1)).astype(np.float32)
    return np.ascontiguousarray(out.reshape(B_FULL, C, H, W))
